# revision 82
# baseline (speedup 1.0000x reference)
"""Multi-head causal attention (B=2, S=2048, HID=2048, H=16, D=128) on 8 TRN2
NeuronCores.

Sharding: core c handles batch b=c//4 and heads [4*(c%4) .. 4*(c%4)+3].
Each core computes qkv-projection + RoPE + causal attention + its partial
out-projection; the host sums the 4 partial outputs per batch (tensor-parallel
reduce) and stacks the 2 batches.

v9 (from v3..v8): PE kept continuously fed (TRN2 drops to the 1.2GHz p-state
after any >100ns stall):
 - softmax row-sums off the PE: A tiles are per-QUAD [128, 2 units, 2
   chunks, QB]; one wide DVE add pair-sums two units (bf16 2x), an acc2
   chain accumulates per-block, and a 2-pass ones-matmul per (h, q-block)
   does the final cross-partition reduce (was nkc passes).
 - diagonal pairs: single merged exp from a common column offset; one
   right-aligned [zeros|tri] mask slice per chunk on the idle GpSimd zeroes
   dead + triangle columns (tensor_tensor only — Q7 lib swaps cost ~4us,
   so no gpsimd memsets in the attention phase; the A ring is pre-zeroed
   once during phase 1 against stale-NaN x0).
 - attention is q-block-major; q-block 0's scores/exp/mask/pairsum are
   PRE-EMITTED into the phase-1 tail (PSUM: psQK 2 + psV 2 + psS 4 banks),
   so the attention phase opens with ready AV matmuls instead of an
   exp-paced spin-up.
 - the out-projection is interleaved into the next q-block's unit stream;
   block finalize (R + normalize) lags 2 units behind the last AV.
 - rope in bf16 end-to-end (DVE 2x, half rot-DMA traffic).
 - dual-queue DMA (SP + Activation DGE) ordered by first use; late-phase
   loads (x3, w_out, masks) issue mid-phase-1 where they cost nothing.
 - warmup + zeros-stationary dummy matmuls accumulate +0 into live PSUM
   groups to hold the PE clock through the DMA-paced start.

On-chip layout: activations kept transposed ([feature, token]):
  qT/kT = W_qk^T-slice @ x^T   (RoPE applied during PSUM evacuation)
  S^T[k,q] = kT^T@qT ; A = exp(S^T*scale) (*causal mask on the diagonal)
  outT[d,q] = V^T-chunks @ A   (accumulated over k chunks)
  y[tok,col] = outT^T-chunks @ W_o-rows  (accumulated over heads)
"""
import sys

sys.path.insert(0, '/opt/trn_rl_repo')

import numpy as np
import ml_dtypes

B, S, HID = 2, 2048, 2048
H, D = 16, 128
NH = H // 4          # heads per core = 4
HC = HID // 128      # hid chunks = 16
TB = 512             # token block for projection
NTB = S // TB        # 4
QB = 512             # q block in attention
NQB = S // QB        # 4
NKCH = S // 128      # k chunks total = 16
SCALE = 1.0 / float(np.sqrt(D))
BASE = 10000.0
N_CORES = 8

_cache = {}


def _build():
    import concourse.bass as bass  # noqa: F401
    import concourse.tile as tile
    from concourse import bacc, mybir

    f32 = mybir.dt.float32
    bf16 = mybir.dt.bfloat16
    EXP = mybir.ActivationFunctionType.Exp
    MULT = mybir.AluOpType.mult
    ADD = mybir.AluOpType.add

    nc = bacc.Bacc("TRN2", target_bir_lowering=False, debug=False,
                   num_devices=N_CORES)

    xT = nc.dram_tensor("xT", [HID, S], bf16, kind="ExternalInput").ap()
    wqk = nc.dram_tensor("wqk", [HID, 2 * NH * D], bf16, kind="ExternalInput").ap()
    wv = nc.dram_tensor("wv", [HID, NH * D], bf16, kind="ExternalInput").ap()
    wo = nc.dram_tensor("wo", [NH * D, HID], bf16, kind="ExternalInput").ap()
    cosT = nc.dram_tensor("cosT", [D, S], bf16, kind="ExternalInput").ap()
    sinS = nc.dram_tensor("sinS", [D, S], bf16, kind="ExternalInput").ap()
    # trid[k, q] = [zeros(384) | (q >= k) triangle(128)]: right-aligned
    # suffix slices mask a diagonal chunk's dead + triangle columns
    tridM = nc.dram_tensor("tridM", [128, 512], bf16, kind="ExternalInput").ap()
    ones_sq = nc.dram_tensor("ones_sq", [128, 128], bf16, kind="ExternalInput").ap()
    y = nc.dram_tensor("y", [S, HID], bf16, kind="ExternalOutput").ap()
    warm = nc.dram_tensor("warm", [128, 8], f32, kind="ExternalOutput").ap()

    with tile.TileContext(nc) as tc:
      with tc.tile_pool(name="persist", bufs=1) as pp, \
           tc.tile_pool(name="p23w", bufs=1) as p2w, \
           tc.tile_pool(name="p2c", bufs=1) as p2c:
        # per-(col, jb) qk tiles and per-chunk v tiles → fine-grained deps
        qkT = [[pp.tile([128, TB], bf16, tag=f"qkT{i}_{j}",
                        name=f"qkT{i}_{j}") for j in range(NTB)]
               for i in range(8)]
        v_t = [pp.tile([128, NH * D], bf16, tag=f"v{cg}", name=f"v{cg}")
               for cg in range(NKCH)]
        # attention/out-projection persistents, hoisted so their DMAs can
        # issue during phase 1 instead of at the phase boundary
        wot = [p2w.tile([128, HID], bf16, tag=f"wot{h}", name=f"wot{h}")
               for h in range(NH)]
        outT = [p2w.tile([128, S], bf16, tag=f"outT{h}", name=f"outT{h}")
                for h in range(NH)]
        trid = p2c.tile([128, 512], bf16, tag="trid")
        t1s = p2c.tile([128, 128], bf16, tag="t1s")
        z128 = p2c.tile([128, 128], bf16, tag="z128")
        p2 = tc.alloc_tile_pool(name="p2", bufs=4)   # A-quad ring
        pacc = tc.alloc_tile_pool(name="p2acc", bufs=2)
        # S PSUM hoisted: q-block 0's scores+exp+mask+pairsum are
        # pre-emitted into the phase-1 tail, so the attention phase opens
        # with its AV matmuls (no exp-paced boundary spin-up)
        psS = tc.alloc_tile_pool(name="psS", bufs=2, space="PSUM")

        # flat unit list, q-block-major: all of q-block jb4 (across heads)
        # completes before jb4+1, so the out-projection for jb4's tokens
        # can interleave into jb4+1's unit stream
        units = []
        for jb4 in range(NQB):
            for h in range(NH):
                nkc = (QB // 128) * (jb4 + 1)
                for kp in range(nkc // 2):
                    units.append((h, jb4, kp, nkc))
        N_PRE = 8   # jb4=0's units, pre-emitted into the phase-1 tail

        blocks = {}   # (h, jb4) -> dict with O, Aq, acc2, A-views

        def emit_scores(unit):
            """S-pair matmuls + merged exp (+ diag masks on GpSimd) + quad
            pair-sum / acc2 chain on DVE for one unit."""
            h, jb4, kp, nkc = unit
            kc0 = 2 * kp
            kT = qkT[NH + h]
            qT_b = qkT[h][jb4]
            blk = blocks[(h, jb4)]
            p = kp % 2
            if p == 0:  # new quad tile: [part, unit, chunk, q]
                # the diagonal quad (last of each block) gets its own ring:
                # its dead columns are never written by exp or mask, so they
                # stay zero from a single pre-zero forever, and the masks
                # can cover only the triangle + freshly-computed region
                # Adiag bufs=4: all four pre-emitted jb4=0 blocks are diag
                # quads whose AVs only run in the attention phase — fewer
                # buffers would let a later pre-emit overwrite an earlier
                # block's tile before its AV is even emitted
                tag = "Adiag" if kp == 2 * jb4 else "A"
                blk["Aq"] = p2.tile([128, 2, 2, QB], bf16, tag=tag,
                                    name="A", bufs=2 if tag == "A" else 4)
            Aq = blk["Aq"]
            A = Aq[:, p]                 # [128, 2, QB]
            Sc = psS.tile([128, 2, QB], f32, tag="S", name="S")
            md = kc0 - (QB // 128) * jb4
            if md < 0:  # fully below the diagonal: full pair
                for i in range(2):
                    kc = kc0 + i
                    nc.tensor.matmul(
                        Sc[:, i, :],
                        kT[kc // 4][:, (kc % 4) * 128:(kc % 4 + 1) * 128],
                        qT_b[:], start=True, stop=True)
                nc.scalar.activation(A[:, :, :], Sc[:, :, :], EXP,
                                     scale=SCALE)
            else:  # diagonal pair
                off0 = md * 128          # 0 or 256
                # both chunks computed from off0 (chunk 1 gets 128 extra
                # non-causal cols, squashed by the mask)
                for i in range(2):
                    kc = kc0 + i
                    nc.tensor.matmul(
                        Sc[:, i, off0:],
                        kT[kc // 4][:, (kc % 4) * 128:(kc % 4 + 1) * 128],
                        qT_b[:, off0:], start=True, stop=True)
                nc.scalar.activation(A[:, :, off0:], Sc[:, :, off0:], EXP,
                                     scale=SCALE)
                # one suffix-slice of trid per chunk zeroes the triangle +
                # the freshly-computed non-causal cols ([off0:off0+mw); the
                # cols below off0 are never written in an Adiag tile and
                # stay zero from the one-time pre-zero). DVE, not GpSimd: a
                # Pool tensor_tensor takes ~1.2us on the exp→AV critical
                # path; DVE does it in ~0.4us, in-order with the pair-sum
                for i in range(2):
                    mw = (i + 1) * 128       # 128 or 256
                    nc.vector.tensor_tensor(
                        out=A[:, i, off0:off0 + mw],
                        in0=A[:, i, off0:off0 + mw],
                        in1=trid[:, 512 - mw:], op=MULT)
            blk.setdefault("A", {})[kp] = A
            if p == 1:  # quad complete: one wide pair-sum on DVE
                if kp == 1:
                    acc2 = pacc.tile([128, 2, QB], bf16, tag="acc2",
                                     name="acc2")
                    blk["acc2"] = acc2
                    nc.vector.tensor_tensor(
                        out=acc2[:], in0=Aq[:, :, 0, :], in1=Aq[:, :, 1, :],
                        op=ADD)
                else:
                    t2 = pacc.tile([128, 2, QB], bf16, tag="t2", name="t2",
                                   bufs=1)
                    nc.vector.tensor_tensor(
                        out=t2[:], in0=Aq[:, :, 0, :], in1=Aq[:, :, 1, :],
                        op=ADD)
                    nc.vector.tensor_tensor(
                        out=blk["acc2"][:], in0=blk["acc2"][:], in1=t2[:],
                        op=ADD)

        def pre_emit(lo, hi):
            for u in units[lo:hi]:
                if u[2] == 0:
                    blocks[(u[0], u[1])] = {}
                emit_scores(u)

        # ---- phase 1: fused V + QK projection (x and weights loaded once)
        with tc.tile_pool(name="p1w", bufs=1) as p1w, \
             tc.tile_pool(name="p1x", bufs=2) as p1x, \
             tc.tile_pool(name="rope", bufs=2) as rp, \
             tc.tile_pool(name="trig", bufs=1) as tp, \
             tc.tile_pool(name="psQK", bufs=2, space="PSUM") as psq, \
             tc.tile_pool(name="psV", bufs=2, space="PSUM") as psv:
            # warmup stationary via GpSimd memset — ready ~3us before a DMA
            # could land it. Also pre-loads the Q7 memset+tensor_tensor
            # ucode libs (a lib switch costs ~4us) and pre-zeroes the
            # A-quad ring (dead cols are only masked x0 after exp; stale
            # SBUF bits there could decode as NaN, and NaN*0=NaN).
            wones = p1w.tile([128, 128], bf16, tag="wones")
            nc.gpsimd.memset(wones[:], 1.0)
            nc.gpsimd.memset(z128[:], 0.0)
            # warmup burst holds the PE through the DMA-bound start; it
            # shares the QK P-ring (slot recycled), and the warm export is
            # read out right away while ACT is idle
            Pw = psq.tile([128, 128], f32, tag="P", name="Pw")
            for _ in range(56):
                nc.tensor.matmul(Pw[:], wones[:], wones[:],
                                 start=True, stop=True)
            wsb = p1w.tile([128, 8], f32, tag="wsb")
            nc.scalar.copy(wsb[:], Pw[:, 0:8])
            nc.sync.dma_start(warm, wsb[:])

            # per-chunk weight and x tiles: the first matmuls can start as
            # soon as their own 128-row chunk has landed
            wv_c = [p1w.tile([128, NH * D], bf16, tag=f"wv{c}",
                             name=f"wv{c}") for c in range(HC)]
            wq_c = [p1w.tile([128, 2 * NH * D], bf16, tag=f"wq{c}",
                             name=f"wq{c}") for c in range(HC)]
            xc_t = [[p1x.tile([128, TB], bf16, tag=f"xc{c}",
                              name=f"xc{jb}_{c}") for c in range(HC)]
                    for jb in range(NTB)]

            def dma_x(jb, c, eng=None):
                (eng or nc.sync).dma_start(
                    xc_t[jb][c][:],
                    xT[c * 128:(c + 1) * 128, jb * TB:(jb + 1) * TB])

            # triple-queue DMA by first-use priority: SP (sync) and
            # Activation (scalar) hardware DGEs fill the start-critical
            # wv+x0+trig+wq; a dma_start costs ~600ns of issuing-engine
            # time, so everything needed later (x1-x3, consts, w_out) is
            # issued from the otherwise-idle GpSimd software DGE.
            tcos = tp.tile([D, S], bf16, tag="tcos")
            tsin = tp.tile([D, S], bf16, tag="tsin")
            # projection order is v0 v1 qk0 qk1 ..., so the queues load
            # x0+wv+x1 (V work: 3.4us/MB) first and wq (QK: needed only
            # from ~35us) behind them — queue FIFO allocates the ~360GB/s
            # of HBM bandwidth in exactly that priority
            for c in range(HC):
                nc.scalar.dma_start(wv_c[c][:], wv[c * 128:(c + 1) * 128, :])
                dma_x(0, c)
            for c in range(HC):
                dma_x(1, c)
            nc.gpsimd.dma_start(tcos[:], cosT)  # rope needs trig ~37us
            nc.gpsimd.dma_start(tsin[:], sinS)
            for c in range(HC):
                q = nc.gpsimd if c < HC // 2 else nc.sync
                q.dma_start(wq_c[c][:], wqk[c * 128:(c + 1) * 128, :])

            def v_proj(jb):
                xc = xc_t[jb]
                for t2 in range(TB // 128):
                    cg = jb * (TB // 128) + t2  # 128-token chunk
                    Pv = psv.tile([128, NH * D], f32, tag="Pv", name="Pv")
                    for c in range(HC):
                        nc.tensor.matmul(
                            Pv[:],
                            xc[c][:, t2 * 128:(t2 + 1) * 128],
                            wv_c[c][:],
                            start=(c == 0), stop=(c == HC - 1))
                        if jb <= 1 and t2 == 0 and 0 < c < HC - 1:
                            # first groups are DMA-paced (~0.7us/chunk):
                            # zeros-stationary dummies accumulate +0 into
                            # the open Pv group, keeping the PE clock
                            # ramped through the arrival gaps
                            for _ in range(6 if jb == 0 else 3):
                                nc.tensor.matmul(
                                    Pv[:, 0:128], z128[:], wones[:],
                                    start=False, stop=False,
                                    skip_group_check=True)
                    nc.scalar.copy(v_t[cg][:], Pv[:])

            def qk_proj(jb):
                xc = xc_t[jb]
                sl = slice(jb * TB, (jb + 1) * TB)
                for cc in range(8):  # 4 q cols then 4 k cols
                    P = psq.tile([128, TB], f32, tag="P", name="P")
                    for c in range(HC):
                        nc.tensor.matmul(
                            P[:],
                            wq_c[c][:, cc * 128:(cc + 1) * 128],
                            xc[c][:],
                            start=(c == 0), stop=(c == HC - 1))
                    # rope in bf16 end-to-end: half the DVE cost (2x mode)
                    # and half the rot-DMA traffic. The rot swaps issue on
                    # the scalar DGE — on sync they would sit behind the
                    # bulk x prefetches in the queue FIFO
                    u = rp.tile([128, TB], bf16, tag="u", name="u")
                    nc.scalar.copy(u[:], P[:])
                    rot = rp.tile([128, TB], bf16, tag="rot", name="rot")
                    nc.scalar.dma_start(rot[0:64, :], u[64:128, :])
                    nc.scalar.dma_start(rot[64:128, :], u[0:64, :])
                    t = rp.tile([128, TB], bf16, tag="t", name="t")
                    nc.vector.tensor_tensor(
                        out=t[:], in0=u[:], in1=tcos[:, sl], op=MULT)
                    m = rp.tile([128, TB], bf16, tag="m", name="m")
                    nc.vector.tensor_tensor(
                        out=m[:], in0=rot[:], in1=tsin[:, sl], op=MULT)
                    nc.vector.tensor_tensor(
                        out=qkT[cc][jb][:], in0=t[:], in1=m[:], op=ADD)

            # q-block-major attention needs jb=0 first and jb=3 last; each
            # rope tail (DVE) drains behind the following matmul work, and
            # jb4=0's scores pre-emit into the tail as their inputs land.
            # Adiag-ring pre-zero + TT lib warm go behind the start-critical
            # gpsimd DMA issues (each memset is ~1.8us of Pool time). The
            # full-quad "A" ring needs no pre-zero: every generation is
            # fully overwritten by its exp before any read.
            for _ in range(4):
                az = p2.tile([128, 2, 2, QB], bf16, tag="Adiag", name="A",
                             bufs=4)
                nc.gpsimd.memset(az[:, :, :, :], 0.0)
            nc.gpsimd.tensor_tensor(out=z128[:, 0:8], in0=z128[:, 0:8],
                                    in1=z128[:, 0:8], op=MULT)
            v_proj(0)
            qk_proj(0)
            v_proj(1)
            qk_proj(1)
            # x2/x3 + late-phase loads on the GpSimd software DGE: their
            # descriptors wait for qk(0)/qk(1)'s last xc reads; none may
            # block jb2's rot DMAs (sync) or cost the Activation engine
            for c in range(HC):
                dma_x(2, c, eng=nc.gpsimd)
            v_proj(2)
            pre_emit(0, 2)
            for c in range(HC):
                dma_x(3, c, eng=nc.gpsimd)
            nc.gpsimd.dma_start(trid[:], tridM[:])
            nc.gpsimd.dma_start(t1s[:], ones_sq[:])
            for h in range(NH):
                nc.gpsimd.dma_start(wot[h][:], wo[h * 128:(h + 1) * 128, :])
            qk_proj(2)
            pre_emit(2, 4)
            qk_proj(3)
            pre_emit(4, 6)
            v_proj(3)
            pre_emit(6, 8)

        # ---- phases 2+3: attention with interleaved out-projection ----
        with tc.tile_pool(name="p2r", bufs=2) as p2r, \
             tc.tile_pool(name="p3s", bufs=4) as p3s, \
             tc.tile_pool(name="psO", bufs=2, space="PSUM") as psO, \
             tc.tile_pool(name="psRP3", bufs=2, space="PSUM") as psmix:

            def emit_av(unit):
                """AV matmuls for one unit (DEPTH units behind)."""
                h, jb4, kp, nkc = unit
                blk = blocks[(h, jb4)]
                if kp == 0:
                    blk["O"] = psO.tile([128, QB], f32, tag="O", name="O")
                A = blk["A"].pop(kp)         # [128, 2, QB]
                kc0 = 2 * kp
                md = kc0 - (QB // 128) * jb4
                for i in range(2):
                    kc = kc0 + i
                    off = max(0, (md + i) * 128)
                    nc.tensor.matmul(
                        blk["O"][:, off:],
                        v_t[kc][:, h * D:(h + 1) * D],
                        A[:, i, off:],
                        start=(kc == 0), stop=(kc == nkc - 1),
                        skip_group_check=True)

            def emit_finalize(h, jb4):
                """Rowsum reduce + normalize, emitted a couple of units
                after the block's last AV so the DVE acc2 chain has
                drained — the R matmul then doesn't head-of-line-block the
                in-order PE queue."""
                blk = blocks[(h, jb4)]
                R = psmix.tile([128, QB], f32, tag="RP3", name="R")
                acc2 = blk["acc2"]
                for p_ in range(2):
                    nc.tensor.matmul(R[:], t1s[:], acc2[:, p_, :],
                                     start=(p_ == 0), stop=(p_ == 1),
                                     skip_group_check=True)
                qsl = slice(jb4 * QB, (jb4 + 1) * QB)
                rc = p2r.tile([128, QB], f32, tag="rc", name="rc")
                nc.vector.reciprocal_approx_fast(rc[:], R[:])
                nc.vector.tensor_tensor(
                    out=outT[h][:, qsl], in0=blk["O"][:], in1=rc[:], op=MULT)

            def emit_p3(tch, cbp):
                """One out-projection piece: [128 tokens, 1024 y cols]."""
                ys = p3s.tile([128, 1024], bf16, tag="ys", name="ys")
                for cb2 in range(2):
                    cb = cbp * 2 + cb2
                    P3 = psmix.tile([128, 512], f32, tag="RP3", name="P3")
                    for h in range(NH):
                        nc.tensor.matmul(
                            P3[:],
                            outT[h][:, tch * 128:(tch + 1) * 128],
                            wot[h][:, cb * 512:(cb + 1) * 512],
                            start=(h == 0), stop=(h == NH - 1))
                    # all evacs on DVE: the scalar engine (exp) is the
                    # attention bottleneck; the mask shrink freed DVE room
                    dst = ys[:, cb2 * 512:(cb2 + 1) * 512]
                    nc.vector.tensor_copy(dst, P3[:])
                nc.sync.dma_start(
                    y[tch * 128:(tch + 1) * 128,
                      cbp * 1024:(cbp + 1) * 1024], ys[:])

            # schedules: p3 pieces for q-block jb4 spread over the deeper
            # 2/3 of block jb4+1; finalize lags 2 units behind last AV
            block_start = {}
            idx0 = 0
            for jb4 in range(NQB):
                block_start[jb4] = idx0
                idx0 += NH * (2 * (jb4 + 1))
            n_units = idx0
            DEPTH = 2
            FIN_LAG = 2
            p3_sched = {}   # loop idx -> list of (tch, cbp)
            for jb4 in range(NQB - 1):
                start = block_start[jb4 + 1]
                end = block_start[jb4 + 2] if jb4 + 2 in block_start \
                    else n_units
                first = start + max(DEPTH, (end - start) // 3)
                slots = list(range(first, end))
                pieces = [(4 * jb4 + t, cbp)
                          for t in range(4) for cbp in range(2)]
                step = len(slots) / len(pieces)
                for j, pc in enumerate(pieces):
                    p3_sched.setdefault(slots[int(j * step)], []).append(pc)
            fin_sched = {}
            e = 0
            for jb4 in range(NQB):
                for h in range(NH):
                    e += 2 * (jb4 + 1)
                    # jb4=3's longer acc2 chains need one more unit of lag
                    lag = FIN_LAG + (1 if jb4 == 3 else 0)
                    fin_sched.setdefault(e - 1 + DEPTH + lag,
                                         []).append((h, jb4))

            for idx, unit in enumerate(units):
                h, jb4, kp, nkc = unit
                if idx >= N_PRE:
                    if kp == 0:
                        blocks[(h, jb4)] = {}
                    emit_scores(unit)
                if idx >= DEPTH:
                    emit_av(units[idx - DEPTH])
                for hb in fin_sched.get(idx, ()):
                    emit_finalize(*hb)
                for (tch, cbp) in p3_sched.get(idx, ()):
                    emit_p3(tch, cbp)
            for vidx in range(len(units), max(fin_sched) + 1):
                if vidx - DEPTH < len(units):
                    emit_av(units[vidx - DEPTH])
                for hb in fin_sched.get(vidx, ()):
                    emit_finalize(*hb)
            for t in range(4):
                for cbp in range(2):
                    emit_p3(12 + t, cbp)
        psS.release()
        pacc.release()
        p2.release()

    nc.compile()
    return nc


def _host_inputs(x, w_qkv, w_out):
    """Build the 8 per-core input maps."""
    bf16 = ml_dtypes.bfloat16
    # RoPE tables, transposed ([d, t]) with the rotate-half sign folded in.
    inv_freq = 1.0 / (BASE ** (np.arange(0, D, 2, dtype=np.float64) / D))
    pos = np.arange(S, dtype=np.float64)
    freqs = np.outer(inv_freq, pos)           # [64, S]
    cos_h = np.cos(freqs).astype(np.float32)
    sin_h = np.sin(freqs).astype(np.float32)
    cosT = np.concatenate([cos_h, cos_h], 0).astype(bf16)   # [128, S]
    sinS = np.concatenate([-sin_h, sin_h], 0).astype(bf16)  # signed sin

    # [zeros(384) | 128x128 causal triangle] ([k-part, q-free]): q >= k live
    kp = np.arange(128)[:, None]
    qf = np.arange(128)[None, :]
    tridM = np.concatenate(
        [np.zeros((128, 384), np.float32), (qf >= kp).astype(np.float32)],
        axis=1).astype(bf16)

    w3 = np.asarray(w_qkv, np.float32).reshape(HID, 3, H, D)
    wo_full = np.asarray(w_out, np.float32).reshape(H, D, HID)
    x = np.asarray(x, np.float32)
    xT_b = [np.ascontiguousarray(x[b].T).astype(bf16) for b in range(B)]

    shared = {
        "cosT": cosT, "sinS": sinS, "tridM": tridM,
        "ones_sq": np.ones((128, 128), bf16),
    }
    in_maps = []
    for c in range(N_CORES):
        b, hg = c // 4, c % 4
        heads = slice(4 * hg, 4 * hg + 4)
        wqk = np.ascontiguousarray(
            w3[:, 0:2, heads, :].reshape(HID, 2 * NH * D)).astype(bf16)
        wv = np.ascontiguousarray(
            w3[:, 2, heads, :].reshape(HID, NH * D)).astype(bf16)
        wo_c = np.ascontiguousarray(
            wo_full[heads].reshape(NH * D, HID)).astype(bf16)
        in_maps.append({
            "xT": xT_b[b],
            "wqk": wqk, "wv": wv, "wo": wo_c, **shared,
        })
    return in_maps


def kernel(x, w_qkv, w_out):
    from concourse.bass_utils import run_bass_kernel_spmd

    if "nc" not in _cache:
        _cache["nc"] = _build()
    nc = _cache["nc"]
    in_maps = _host_inputs(x, w_qkv, w_out)
    res = run_bass_kernel_spmd(nc, in_maps, core_ids=list(range(N_CORES)))
    out = np.zeros((B, S, HID), np.float32)
    for c in range(N_CORES):
        out[c // 4] += res.results[c]["y"].astype(np.float32)
    return out
